# revision 1
# baseline (speedup 1.0000x reference)
"""GNN message-passing layer (GSS GNNLayer) on 8 Trainium2 NeuronCores.

Math (see reference):
    Ax   = A @ x                 (sparse COO, E edges)
    pre1 = Ax @ W1.T + b1
    Axx  = A @ (Ax * x)
    pre2 = Axx @ W2.T + b2
    pre  = pre1 + pre2 ; out = elu(pre) ; return (pre, out)

Distribution: row-partition by destination node; core c owns dest rows
[c*5000, (c+1)*5000). Edges are bucketed by (core, dest-block of 128,
lo/hi source-table half) on the host, sorted by source within a bucket
(HBM locality), and padded to chunks of 128 with val=0 edges.

SpMM: per chunk of 128 edges the device dma_gather's the 128 source
rows from a packed split-precision table row [bf16(x) | bf16(x-bf16 x)]
(512 B/row, int16 indices, table halved at row 32768, 4 SWDGE queues
round-robin), builds two bf16 selection matrices
    S_vh[e,d] = val_hi[e] * (d == rowlocal[e])
    S_vl[e,d] = val_lo[e] * (d == rowlocal[e])
with fused DVE tensor_scalar ops, and accumulates
    S_vh.T @ M_hi + S_vl.T @ M_hi + S_vh.T @ M_lo
into the dest block's fp32 PSUM tile (the dropped vl*lo term is
~2^-18 relative). val_hi/val_lo are the host's bf16 split of val.

Between the passes H = Ax*x is split the same way on device and
AllGather'd. Dense finals per block: TensorE transpose + two fp32
matmuls with host-transposed W1T/W2T (rotating weight-tile copies;
a single shared stationary tile measures ~3.5us/matmul), bias add,
ELU = max(x,0) + exp(min(x,0)) - 1.

SPMD: one program for all 8 cores; per-(block,table) chunk counts are
the max over cores, computed from the actual input, so the program
structure is uniform and only the data differs.
"""

import os
import numpy as np
import ml_dtypes

BF16 = ml_dtypes.bfloat16

N = 40000
D = 128
E = 640000
NCORES = 8
NSH = N // NCORES          # 5000 dest rows per core
P = 128
NB = (NSH + P - 1) // P    # 40 dest blocks per core (last has 8 rows)
SBW = 2                    # blocks per gather super-block
SPLIT = 32768              # int16 gather index limit
NQ = 4                     # SWDGE queues for gathers

_cache = {}


def _preprocess(adj_row, adj_col, adj_val):
    """Bucket/pad edges; build per-core gather-index and S-descriptor arrays."""
    row = np.asarray(adj_row, np.int64)
    col = np.asarray(adj_col, np.int64)
    val = np.asarray(adj_val, np.float32)

    core = row // NSH
    loc = row - core * NSH
    blk = loc // P
    dloc = (loc % P).astype(np.float32)
    hi = (col >= SPLIT).astype(np.int64)

    key = (core * NB + blk) * 2 + hi          # 0 .. NCORES*NB*2-1
    nkey = NCORES * NB * 2
    order = np.lexsort((col, key))            # bucket-major, source-sorted
    sk = key[order]
    counts = np.bincount(key, minlength=nkey)
    gstart = np.concatenate([[0], np.cumsum(counts)[:-1]])
    pos = np.arange(len(sk)) - gstart[sk]     # rank within its bucket

    cnt = counts.reshape(NCORES, NB, 2)
    caps = np.ceil(cnt / P).astype(np.int64).max(axis=0)   # [NB, 2]
    caps[:, 0] = np.maximum(caps[:, 0], 1)    # every block needs >=1 chunk
    caps_lo = caps[:, 0]
    caps_hi = caps[:, 1]

    # chunk-column layout: per block, lo chunks then hi chunks
    col0 = np.zeros((NB, 2), np.int64)
    run = 0
    for b in range(NB):
        col0[b, 0] = run
        run += caps_lo[b]
        col0[b, 1] = run
        run += caps_hi[b]
    TC = int(run)
    lostart = np.concatenate([[0], np.cumsum(caps_lo)])
    histart = np.concatenate([[0], np.cumsum(caps_hi)])
    CL = int(lostart[-1]) * 8                 # idx cols (16 idx/col)
    CH = max(int(histart[-1]) * 8, 1)

    rowloc = np.zeros((NCORES, P, TC), np.float32)
    vhi = np.zeros((NCORES, P, TC), np.float32)
    vlo = np.zeros((NCORES, P, TC), np.float32)
    vratio = np.zeros((NCORES, P, TC), np.float32)
    idxlo = np.zeros((NCORES, P, CL), np.int16)
    idxhi = np.zeros((NCORES, P, CH), np.int16)

    cS = sk // (NB * 2)
    bS = (sk // 2) % NB
    tS = sk % 2
    dS = dloc[order]
    vS = val[order]
    colS = col[order]
    vh = vS.astype(BF16)
    vl = (vS - vh.astype(np.float32)).astype(BF16)

    ccol = col0[bS, tS] + pos // P
    pp = pos % P
    rowloc[cS, pp, ccol] = dS
    vhi[cS, pp, ccol] = vh.astype(np.float32)
    vlo[cS, pp, ccol] = vl.astype(np.float32)
    vhf = vh.astype(np.float32)
    with np.errstate(divide="ignore", invalid="ignore"):
        rr = np.where(vhf != 0.0, vl.astype(np.float32) / vhf, 0.0)
    vratio[cS, pp, ccol] = rr

    reps = 16 * np.arange(8)[None, :]
    m = tS == 0
    q = lostart[bS[m]] * P + pos[m]
    idxlo[cS[m][:, None], (q % 16)[:, None] + reps, (q // 16)[:, None]] = \
        colS[m].astype(np.int16)[:, None]
    m = tS == 1
    if m.any():
        q = histart[bS[m]] * P + pos[m]
        idxhi[cS[m][:, None], (q % 16)[:, None] + reps, (q // 16)[:, None]] = \
            (colS[m] - SPLIT).astype(np.int16)[:, None]

    return dict(caps_lo=tuple(int(x) for x in caps_lo),
                caps_hi=tuple(int(x) for x in caps_hi),
                TC=TC, CL=CL, CH=CH,
                rowloc=rowloc, vhi=vhi, vlo=vlo, vratio=vratio,
                idxlo=idxlo, idxhi=idxhi)


def _build(caps_lo, caps_hi, TC, CL, CH, reps=1):
    ABL = set(os.environ.get('ABL', '').split(','))
    import concourse.bacc as bacc
    import concourse.mybir as mybir
    import concourse.tile as tile
    from concourse.masks import make_identity

    f32 = mybir.dt.float32
    bf16 = mybir.dt.bfloat16
    i16 = mybir.dt.int16
    Alu = mybir.AluOpType
    Act = mybir.ActivationFunctionType

    lostart = np.concatenate([[0], np.cumsum(caps_lo)]).astype(int)
    histart = np.concatenate([[0], np.cumsum(caps_hi)]).astype(int)
    col0 = np.zeros((NB, 2), np.int64)
    run = 0
    for b in range(NB):
        col0[b, 0] = run
        run += caps_lo[b]
        col0[b, 1] = run
        run += caps_hi[b]

    nc = bacc.Bacc(None, target_bir_lowering=False, num_swdge_queues=NQ)
    x2 = nc.declare_dram_parameter("x2tab", [N, 2 * D], bf16, isOutput=False)
    xsh = nc.declare_dram_parameter("xshard", [NSH, D], f32, isOutput=False)
    idxlo_d = nc.declare_dram_parameter("idxlo", [P, CL], i16, isOutput=False)
    idxhi_d = nc.declare_dram_parameter("idxhi", [P, CH], i16, isOutput=False)
    rowloc_d = nc.declare_dram_parameter("rowloc", [P, TC], f32, isOutput=False)
    vhi_d = nc.declare_dram_parameter("vhi", [P, TC], f32, isOutput=False)
    vlo_d = nc.declare_dram_parameter("vlo", [P, TC], f32, isOutput=False)
    w1t_d = nc.declare_dram_parameter("w1t", [D, D], f32, isOutput=False)
    w2t_d = nc.declare_dram_parameter("w2t", [D, D], f32, isOutput=False)
    bsum_d = nc.declare_dram_parameter("bsum", [P, D], f32, isOutput=False)
    pre_o = nc.declare_dram_parameter("pre", [NSH, D], f32, isOutput=True)
    elu_o = nc.declare_dram_parameter("eluout", [NSH, D], f32, isOutput=True)
    h2sh = nc.dram_tensor("H2_shard", [NSH, 2 * D], bf16)
    h2full = nc.dram_tensor("H2_full", [N, 2 * D], bf16, addr_space="Shared")

    NSB = (NB + SBW - 1) // SBW
    nlo_sb = [sum(caps_lo[s * SBW:(s + 1) * SBW]) for s in range(NSB)]
    nhi_sb = [sum(caps_hi[s * SBW:(s + 1) * SBW]) for s in range(NSB)]
    NROT = 8

    with tile.TileContext(nc) as tc:
        with (
            tc.tile_pool(name="const", bufs=1) as cpool,
            tc.tile_pool(name="mlo", bufs=4) as mlop,
            tc.tile_pool(name="mhi", bufs=4) as mhip,
            tc.tile_pool(name="sel", bufs=12) as spool,
            tc.tile_pool(name="small", bufs=2) as smp,
            tc.tile_pool(name="psum", bufs=4, space="PSUM") as pseg,
            tc.tile_pool(name="psum2", bufs=2, space="PSUM") as ptp,
            tc.tile_pool(name="psum3", bufs=2, space="PSUM") as ppre,
        ):
            iota_b = cpool.tile([P, P], bf16)
            nc.gpsimd.iota(iota_b[:], pattern=[[1, P]], base=0,
                           channel_multiplier=0,
                           allow_small_or_imprecise_dtypes=True)
            ident = cpool.tile([P, P], f32)
            make_identity(nc, ident[:])
            w1t_t = cpool.tile([D, D], f32)
            nc.sync.dma_start(w1t_t[:], w1t_d[:])
            w2t_t = cpool.tile([D, D], f32)
            nc.sync.dma_start(w2t_t[:], w2t_d[:])
            bsum_t = cpool.tile([P, D], f32)
            nc.sync.dma_start(bsum_t[:], bsum_d[:])
            idxlo_t = cpool.tile([P, CL], i16)
            nc.sync.dma_start(idxlo_t[:], idxlo_d[:])
            idxhi_t = cpool.tile([P, CH], i16)
            nc.sync.dma_start(idxhi_t[:], idxhi_d[:])
            rowloc_t = cpool.tile([P, TC], f32)
            nc.sync.dma_start(rowloc_t[:], rowloc_d[:])
            vhi_t = cpool.tile([P, TC], f32)
            nc.sync.dma_start(vhi_t[:], vhi_d[:])
            vlo_t = cpool.tile([P, TC], f32)
            nc.sync.dma_start(vlo_t[:], vlo_d[:])
            ax_all = cpool.tile([P, NB * P], f32)
            # rotating stationary-weight copies for the dense finals
            w1r, w2r, idr = [], [], []
            for k in range(NROT):
                t1 = cpool.tile([D, D], f32, tag=f"w1r{k}")
                nc.vector.tensor_copy(t1[:], w1t_t[:])
                w1r.append(t1)
                t2 = cpool.tile([D, D], f32, tag=f"w2r{k}")
                nc.vector.tensor_copy(t2[:], w2t_t[:])
                w2r.append(t2)
                t3 = cpool.tile([P, P], f32, tag=f"idr{k}")
                nc.vector.tensor_copy(t3[:], ident[:])
                idr.append(t3)

            qctr = [0]

            def run_once():
                def spmm_pass(tlo, thi, finalize):
                    for s in range(NSB):
                        b0 = s * SBW
                        nlo, nhi = nlo_sb[s], nhi_sb[s]
                        mlo = mlop.tile([P, nlo, 2 * D], bf16, tag="mlo")
                        if 'nogather' in ABL:
                            nc.sync.dma_start(mlo[:, 0, :], x2[0:P, :])
                        else:
                            h1 = nlo // 2
                            for (g0, g1) in ((0, h1), (h1, nlo)):
                                if g1 <= g0:
                                    continue
                                nc.gpsimd.dma_gather(
                                    out_ap=mlo[:, g0:g1, :], in_ap=tlo,
                                    idxs_ap=idxlo_t[:, (lostart[b0] + g0) * 8:(lostart[b0] + g1) * 8],
                                    num_idxs=(g1 - g0) * P,
                                    num_idxs_reg=(g1 - g0) * P,
                                    elem_size=2 * D, single_packet=False,
                                    queue_num=qctr[0] % NQ)
                                qctr[0] += 1
                        if nhi:
                            mhi = mhip.tile([P, nhi, 2 * D], bf16, tag="mhi")
                            if 'nogather' in ABL:
                                nc.sync.dma_start(mhi[:, 0, :], x2[0:P, :])
                            else:
                             nc.gpsimd.dma_gather(
                                out_ap=mhi[:], in_ap=thi,
                                idxs_ap=idxhi_t[:, histart[b0] * 8:(histart[b0] + nhi) * 8],
                                num_idxs=nhi * P, num_idxs_reg=nhi * P,
                                elem_size=2 * D, single_packet=False,
                                queue_num=qctr[0] % NQ)
                            qctr[0] += 1
                        swapT = getattr(finalize, 'wants_T', False)
                        for i in range(min(SBW, NB - b0)):
                            b = b0 + i
                            ps = pseg.tile([P, P], f32, tag="seg")
                            tot = caps_lo[b] + caps_hi[b]
                            done = 0
                            glo = sum(caps_lo[b0:b])
                            ghi = sum(caps_hi[b0:b])
                            for j in range(caps_lo[b] + caps_hi[b]):
                                if j < caps_lo[b]:
                                    c = int(col0[b, 0]) + j
                                    msl = mlo[:, glo + j, :]
                                else:
                                    c = int(col0[b, 1]) + (j - caps_lo[b])
                                    msl = mhi[:, ghi + (j - caps_lo[b]), :]
                                if 'noseg' in ABL:
                                    done += 1
                                    continue
                                svh = spool.tile([P, P], bf16, tag="S")
                                nc.vector.tensor_scalar(
                                    svh[:], iota_b[:],
                                    rowloc_t[:, c:c + 1], vhi_t[:, c:c + 1],
                                    op0=Alu.is_equal, op1=Alu.mult)
                                svl = spool.tile([P, P], bf16, tag="S")
                                nc.vector.tensor_scalar(
                                    svl[:], iota_b[:],
                                    rowloc_t[:, c:c + 1], vlo_t[:, c:c + 1],
                                    op0=Alu.is_equal, op1=Alu.mult)
                                first = done == 0
                                if 'nomm' in ABL:
                                    done += 1
                                    continue
                                if swapT:
                                    nc.tensor.matmul(ps[:], lhsT=msl[:, 0:D],
                                                     rhs=svh[:],
                                                     start=first, stop=False)
                                    nc.tensor.matmul(ps[:], lhsT=msl[:, 0:D],
                                                     rhs=svl[:],
                                                     start=False, stop=False)
                                    nc.tensor.matmul(ps[:], lhsT=msl[:, D:2 * D],
                                                     rhs=svh[:],
                                                     start=False,
                                                     stop=(done == tot - 1))
                                else:
                                    nc.tensor.matmul(ps[:], lhsT=svh[:],
                                                     rhs=msl[:, 0:D],
                                                     start=first, stop=False)
                                    nc.tensor.matmul(ps[:], lhsT=svl[:],
                                                     rhs=msl[:, 0:D],
                                                     start=False, stop=False)
                                    nc.tensor.matmul(ps[:], lhsT=svh[:],
                                                     rhs=msl[:, D:2 * D],
                                                     start=False,
                                                     stop=(done == tot - 1))
                                done += 1
                            finalize(b, ps)

                def fin1(b, ps):
                    if 'nofin' in ABL:
                        return
                    rows = min(P, NSH - b * P)
                    axs = ax_all[:, b * P:(b + 1) * P]
                    if 'nomm' in ABL or 'noseg' in ABL:
                        nc.vector.memset(axs, 0.0)
                    else:
                        nc.vector.tensor_copy(axs, ps[:])
                    xb = smp.tile([P, D], f32, tag="xb")
                    nc.sync.dma_start(xb[:rows, :], xsh[b * P:b * P + rows, :])
                    hb = smp.tile([P, D], f32, tag="hb")
                    nc.vector.tensor_tensor(hb[:rows, :], axs[:rows, :],
                                            xb[:rows, :], op=Alu.mult)
                    h2 = smp.tile([P, 2 * D], bf16, tag="h2")
                    nc.vector.tensor_copy(h2[:rows, 0:D], hb[:rows, :])
                    hh32 = smp.tile([P, D], f32, tag="hh32")
                    nc.vector.tensor_copy(hh32[:rows, :], h2[:rows, 0:D])
                    nc.vector.tensor_tensor(h2[:rows, D:2 * D], hb[:rows, :],
                                            hh32[:rows, :], op=Alu.subtract)
                    nc.sync.dma_start(h2sh[b * P:b * P + rows, :], h2[:rows, :])

                spmm_pass(x2[:SPLIT, :], x2[SPLIT:, :], fin1)

                if 'noag' in ABL:
                    pass
                else:
                 nc.gpsimd.collective_compute(
                    "AllGather", Alu.bypass,
                    replica_groups=[list(range(NCORES))],
                    ins=[h2sh[:]], outs=[h2full[:]])

                def fin2(b, ps):
                    if 'nofin' in ABL:
                        return
                    rows = min(P, NSH - b * P)
                    axxT = smp.tile([P, P], f32, tag="axxT")
                    if 'nomm' in ABL or 'noseg' in ABL:
                        nc.vector.memset(axxT[:], 0.0)
                    else:
                        nc.vector.tensor_copy(axxT[:], ps[:])
                    tp = ptp.tile([P, P], f32, tag="tp")
                    nc.tensor.transpose(tp[:], ax_all[:, b * P:(b + 1) * P],
                                        idr[b % NROT][:])
                    axT = smp.tile([P, P], f32, tag="axT")
                    nc.vector.tensor_copy(axT[:], tp[:])
                    pp2 = ppre.tile([P, P], f32, tag="pre")
                    nc.tensor.matmul(pp2[:], lhsT=axT[:], rhs=w1r[b % NROT][:],
                                     start=True, stop=False)
                    nc.tensor.matmul(pp2[:], lhsT=axxT[:], rhs=w2r[b % NROT][:],
                                     start=False, stop=True)
                    pre_sb = smp.tile([P, P], f32, tag="presb")
                    nc.vector.tensor_tensor(pre_sb[:], pp2[:], bsum_t[:],
                                            op=Alu.add)
                    nc.sync.dma_start(pre_o[b * P:b * P + rows, :],
                                      pre_sb[:rows, :])
                    pos = smp.tile([P, P], f32, tag="pos")
                    nc.vector.tensor_scalar_max(pos[:], pre_sb[:], 0.0)
                    neg = smp.tile([P, P], f32, tag="neg")
                    nc.vector.tensor_scalar_min(neg[:], pre_sb[:], 0.0)
                    ex = smp.tile([P, P], f32, tag="ex")
                    nc.scalar.activation(ex[:], neg[:], Act.Exp)
                    elu = smp.tile([P, P], f32, tag="elu")
                    nc.vector.tensor_tensor(elu[:], pos[:], ex[:], op=Alu.add)
                    nc.vector.tensor_scalar_add(elu[:], elu[:], -1.0)
                    nc.sync.dma_start(elu_o[b * P:b * P + rows, :],
                                      elu[:rows, :])


                if 'p2fromx2' in ABL:
                    fin2.wants_T = True
                    spmm_pass(x2[:SPLIT, :], x2[SPLIT:, :], fin2)
                else:
                    fin2.wants_T = True
                    spmm_pass(h2full[:SPLIT, :], h2full[SPLIT:, :], fin2)

            for _ in range(reps):
                run_once()

    nc.compile()
    return nc


def _get_program(pp, reps=1):
    key = (pp["caps_lo"], pp["caps_hi"], reps, os.environ.get("ABL", ""))
    if key not in _cache:
        _cache[key] = _build(list(pp["caps_lo"]), list(pp["caps_hi"]),
                             pp["TC"], pp["CL"], pp["CH"], reps=reps)
    return _cache[key]


def _in_maps(pp, features, W1, b1, W2, b2):
    feats = np.ascontiguousarray(np.asarray(features, np.float32))
    xh = feats.astype(BF16)
    xl = (feats - xh.astype(np.float32)).astype(BF16)
    x2 = np.ascontiguousarray(np.concatenate([xh, xl], axis=1))  # [N, 256] bf16
    w1t = np.ascontiguousarray(np.asarray(W1, np.float32).T)
    w2t = np.ascontiguousarray(np.asarray(W2, np.float32).T)
    bsum = np.tile((np.asarray(b1, np.float32)
                    + np.asarray(b2, np.float32))[None, :], (P, 1))
    maps = []
    for c in range(NCORES):
        maps.append({
            "x2tab": x2,
            "xshard": feats[c * NSH:(c + 1) * NSH],
            "idxlo": pp["idxlo"][c],
            "idxhi": pp["idxhi"][c],
            "rowloc": pp["rowloc"][c],
            "vhi": pp["vhi"][c],
            "vlo": pp["vlo"][c],
            "w1t": w1t,
            "w2t": w2t,
            "bsum": bsum,
        })
    return maps


def kernel(features, adj_row, adj_col, adj_val, W1, b1, W2, b2):
    from concourse.bass_utils import run_bass_kernel_spmd

    pp = _preprocess(adj_row, adj_col, adj_val)
    nc = _get_program(pp)
    maps = _in_maps(pp, features, W1, b1, W2, b2)
    res = run_bass_kernel_spmd(nc, maps, list(range(NCORES)))
    pre = np.concatenate([res.results[c]["pre"] for c in range(NCORES)], axis=0)
    out = np.concatenate([res.results[c]["eluout"] for c in range(NCORES)], axis=0)
    return (pre, out)

